# revision 54
# baseline (speedup 1.0000x reference)
"""Bidirectional ConvLSTM block for Trainium2 (Bass/Tile), 8-core SPMD.

Problem: x [S=16, B=4, Cin=32, H=128, W=128] f32, Wf/Wb [128, 64, 3, 3],
bf/bb [128].  Output [S, B, 2*Co=64, H, W]: forward ConvLSTM hidden states
concat backward ConvLSTM (run on time-reversed x, not re-flipped).

Sharding: 8 independent recurrences = 2 directions x 4 batch elements.
Core k runs direction d=k//4 on batch b=k%4.  No cross-core communication.

Per-core kernel design:
  - SBUF "act" tile [128 part, 130*132] bf16 (ping/pong): zero-padded
    (H+2)x(W+4) spatial plane per channel.
      partitions  0-31 : x_t   centered at col 1+w
      partitions 32-63 : x_t   at col 2+w (shifted right one column)
      partitions 64-95 : h_{t-1} centered
      partitions 96-127: h_{t-1} shifted
    The shifted copy lets one K=128 matmul cover two conv taps (center tap
    dx=dxo, shifted tap dx=dxo-1) at a single rhs column offset dxo+1.
    3x3 conv => 6 matmul passes per output tile; x and h grouped so step 0
    (h=0) runs K=64 x-only matmuls and never reads the h region (no init).
  - Col-tiled matmuls (tile_position (0,32j), mode 128x32): 4 concurrent
    M=32 matmuls per pass, one per spatial tile, each writing ITS gate's
    slice so each PSUM bank ends up gate-pure:
      bank[g] = [128 part = 32ch x 4 spatial tiles, 512] of gate g.
    All pointwise LSTM math then runs on full 128-partition tiles.
  - c state persistent fp32 [128, 4096]; h (bf16) written once into a
    padded 4-row-chunk buffer hc ([0,w0..w127,0,0,0] per chunk, one leading
    zero col).  hc serves: the y store (1 DMA per half step, bf16, host
    unpacks), the center write-back (3-dim SBUF->SBUF DMA per spatial tile
    b per half step), and the shifted write-back (same DMA reading hc at
    offset -1).  An "early" write-back of group 4's first rows keeps the
    next step's group-3 dy=+1 taps from stalling.
"""

import os
import sys

import numpy as np

for _p in ("/opt/trn_rl_repo", "/root/.axon_site/_ro/trn_rl_repo"):
    if os.path.isdir(_p) and _p not in sys.path:
        sys.path.insert(0, _p)

import ml_dtypes  # noqa: E402
import concourse.bass as bass  # noqa: E402,F401
import concourse.mybir as mybir  # noqa: E402
from concourse import bacc, tile  # noqa: E402
from concourse.bass_utils import run_bass_kernel_spmd  # noqa: E402

F32 = mybir.dt.float32
BF16 = mybir.dt.bfloat16
AF = mybir.ActivationFunctionType

S, B, CIN, H, W = 16, 4, 32, 128, 128
CO = 32
HP, WP = H + 2, W + 4          # 130 x 132 padded plane
PADN = HP * WP                 # 17160
NSP = H * W                    # 16384
NT = 512                       # spatial positions per matmul tile (4 rows)
NTP = 4 * WP                   # 528: padded 4-row chunk
TPG = 4                        # tiles per group (col-tiled together)
GROUPS = NSP // (NT * TPG)     # 8 groups per step; group = 16 image rows
N_CORES = 8

EXPLICIT_LDW = os.environ.get("BICLSTM_LDW", "1") == "1"


def mm_noldw(nc, out, lhsT, rhs, start, stop, tile_position):
    """nc.tensor.matmul clone with ldweights=False (weights pre-loaded)."""
    te = nc.tensor
    kd = {0}
    ifmap_ap = te.lower_ap(rhs.opt(kd), opt=False)
    weights_ap = te.lower_ap(lhsT.opt(kd), opt=False, for_matmul_weights=True)
    out_ap = te.lower_ap(out)
    return te.add_instruction(
        mybir.InstMatmult(
            name=te.bass.get_next_instruction_name(),
            replication_resolution=0,
            replication_shift_amnt=0,
            replication_num_rows=0,
            start_tensor_calc=start,
            stop_tensor_calc=stop,
            ins=[ifmap_ap, weights_ap],
            outs=[out_ap],
            perf_mode=None,
            is_transpose=None,
            ifmap_quant_offset=None,
            weights_quant_offset=None,
            bass_skip_group_check=True,
            tile_position=tile_position,
            tile_size=(128, 32),
            ldweights=False,
        )
    )


def build_kernel(nc, tc, x_ap, w_ap, b_ap, y_ap, z_ap, n_steps):
    ctx_pools = []

    def pool(**kw):
        p = tc.tile_pool(**kw)
        ctx_pools.append(p)
        return p.__enter__()

    const = pool(name="const", bufs=1)
    tmp = pool(name="tmp", bufs=3)
    psum = pool(name="psum", bufs=8, space="PSUM")

    # Persistent tiles
    a0 = const.tile([128, PADN], BF16, tag="act0")
    a1 = const.tile([128, PADN], BF16, tag="act1")
    acts = [a0, a1]
    ctile = const.tile([128, GROUPS * NT], F32, tag="c")
    wsb = const.tile([128, 24 * 32], BF16, tag="w")
    bsb = const.tile([128, 4], F32, tag="bias")
    hc = const.tile([128, GROUPS * NTP + 1], BF16, tag="hc")

    nc.sync.dma_start(wsb[:, :], w_ap)
    nc.sync.dma_start(bsb[:, :], b_ap)

    # --- one-time zero init.  Everything else is either host-padded (x),
    # written fully every step (h interior, hc chunks), or never read.
    # h_0 = 0: zero a0's whole h region via SWDGE DMA (parallel with the
    # sync-queue x/w loads); a1 only needs its h pad rows (fold covers rest).
    RS = 36 * WP  # first ~36 padded rows: what pair 0 reads
    nc.gpsimd.dma_start(a0[64:128, 0:RS], z_ap[:, 0:RS])
    nc.gpsimd.dma_start(a0[64:128, RS:], z_ap[:, RS:])
    nc.gpsimd.memset(a1[64:128, 0:WP], 0.0)
    nc.gpsimd.memset(a1[64:128, (HP - 1) * WP :], 0.0)
    hcb = hc[:, 1:].rearrange("p (k w) -> p k w", w=WP)
    nc.vector.memset(hc[:, 0:1], 0.0)
    nc.vector.memset(hcb[:, :, 0:1], 0.0)
    nc.gpsimd.memset(hcb[:, :, W + 1 :], 0.0)

    def load_x(t):
        # x arrives host-padded, center + shifted copies stacked: one DMA
        nc.sync.dma_start(acts[t % 2][0:64, :], x_ap[t])

    # step-0 x load split so the first pair's rows land first
    nc.sync.dma_start(a0[0:64, 0:RS], x_ap[0, :, 0:RS])
    nc.sync.dma_start(a0[0:64, RS:], x_ap[0, :, RS:])

    # h write-back: 4-row full-width (132) blocks, one 3-dim DMA per tile b.
    # Block (g, b) lands at padded plane rows 16g+4b+1 .. +5, cols 0..132.
    # hcv = center-aligned view, hcs = same data shifted one col right.
    hcv = hc[:, 1:].rearrange("p (g f) -> p g f", g=GROUPS)
    hcs = hc[:, 0 : GROUPS * NTP].rearrange("p (g f) -> p g f", g=GROUPS)
    hcq = hc[:, 1:].rearrange("p (g q w) -> p g q w", g=GROUPS, q=4)

    def h_views(a_nxt):
        vc = a_nxt[64:96, WP : WP + H * WP].rearrange(
            "p (g b f) -> p g b f", g=GROUPS, b=TPG
        )
        vs = a_nxt[96:128, WP : WP + H * WP].rearrange(
            "p (g b f) -> p g b f", g=GROUPS, b=TPG
        )
        return vc, vs

    def h_writeback(a_nxt, g0, g1, excl_b0=False):
        # interior rows 1..129 of the padded plane = 32 blocks of (4 x 132)
        vc, vs = h_views(a_nxt)
        for b in range(TPG):
            gg0 = g0 + 1 if (excl_b0 and b == 0) else g0
            nc.sync.dma_start(
                vc[:, gg0:g1, b, :], hcv[32 * b : 32 * b + 32, gg0:g1, :]
            )
            nc.sync.dma_start(
                vs[:, gg0:g1, b, :], hcs[32 * b : 32 * b + 32, gg0:g1, :]
            )

    for t in range(n_steps):
        a_cur = acts[t % 2]
        a_nxt = acts[(t + 1) % 2]
        ar_cur = a_cur.rearrange("p (r w) -> p r w", r=HP)
        if t + 1 < n_steps:
            load_x(t + 1)

        def pointwise(grp, zb):
            csl = ctile[:, grp * NT : (grp + 1) * NT]
            si = tmp.tile([128, NT], F32, tag="si", name=f"si{t}_{grp}")
            so = tmp.tile([128, NT], F32, tag="so", name=f"so{t}_{grp}")
            tg = tmp.tile([128, NT], F32, tag="tg", name=f"tg{t}_{grp}")
            nc.scalar.activation(si[:, :], zb[0][:, :], AF.Sigmoid, bias=bsb[:, 0:1])
            if t > 0:
                sf = tmp.tile([128, NT], F32, tag="sf", name=f"sf{t}_{grp}")
                nc.scalar.activation(sf[:, :], zb[1][:, :], AF.Sigmoid, bias=bsb[:, 1:2])
            nc.scalar.activation(so[:, :], zb[2][:, :], AF.Sigmoid, bias=bsb[:, 2:3])
            nc.scalar.activation(tg[:, :], zb[3][:, :], AF.Tanh, bias=bsb[:, 3:4])

            if t == 0:
                # c_{-1} = 0: c = sig(i)*tanh(g), no f*c term (ctile uninit)
                nc.vector.tensor_mul(csl, si[:, :], tg[:, :])
            else:
                t2 = tmp.tile([128, NT], F32, tag="t2", name=f"t2_{t}_{grp}")
                t3 = tmp.tile([128, NT], F32, tag="t3", name=f"t3_{t}_{grp}")
                # t3 only needs sf (2nd activation) - run it before t2 so the
                # DVE isn't blocked behind tanh_g on the critical chain
                nc.vector.tensor_mul(t3[:, :], sf[:, :], csl)
                nc.vector.tensor_mul(t2[:, :], si[:, :], tg[:, :])
                nc.vector.tensor_add(csl, t2[:, :], t3[:, :])

            tcn = tmp.tile([128, NT], F32, tag="tcn", name=f"tcn{t}_{grp}")
            nc.scalar.activation(tcn[:, :], csl, AF.Tanh)
            # h in bf16, written into the padded row-chunk buffer
            soq = so[:, :].rearrange("p (q w) -> p q w", q=4)
            tcq = tcn[:, :].rearrange("p (q w) -> p q w", q=4)
            nc.vector.tensor_mul(hcq[:, grp, :, 1 : W + 1], soq, tcq)

        # Group pairs: adjacent groups' matmuls interleaved so same-weight
        # matmuls are consecutive (better PE pipelining).
        for pg in range(GROUPS // 2):
            ga, gb = 2 * pg, 2 * pg + 1
            zbs = {
                grp: [
                    psum.tile([128, NT], F32, tag="z", name=f"z{t}_{grp}_{g}")
                    for g in range(4)
                ]
                for grp in (ga, gb)
            }
            for g in range(4):
                for p in range(6):
                    dy = (p % 3) - 1
                    dxo = 0 if p < 3 else 1
                    col = (g * 6 + p) * 32
                    lhsT = wsb[:, col : col + 32]
                    if EXPLICIT_LDW:
                        for j in range(TPG):
                            nc.tensor.ldweights(lhsT, tile_position=(0, 32 * j))
                    for j in range(TPG):
                        for grp in (ga, gb):
                            r0 = 16 * grp + 4 * j
                            rhs = ar_cur[
                                :, r0 + 1 + dy : r0 + 5 + dy, 1 + dxo : W + 1 + dxo
                            ]
                            if EXPLICIT_LDW:
                                mm_noldw(
                                    nc,
                                    zbs[grp][g][32 * j : 32 * j + 32, :],
                                    lhsT,
                                    rhs,
                                    start=(p == 0),
                                    stop=(p == 5),
                                    tile_position=(0, 32 * j),
                                )
                            else:
                                nc.tensor.matmul(
                                    zbs[grp][g][32 * j : 32 * j + 32, :],
                                    lhsT,
                                    rhs,
                                    start=(p == 0),
                                    stop=(p == 5),
                                    skip_group_check=True,
                                    tile_position=(0, 32 * j),
                                )
            pointwise(ga, zbs[ga])
            pointwise(gb, zbs[gb])
            last = t + 1 == n_steps
            if pg == 1:
                if not last:
                    h_writeback(a_nxt, 0, 4)
                nc.sync.dma_start(y_ap[t, :, 0 : 4 * NTP], hc[:, 1 : 1 + 4 * NTP])
            elif pg == 2:
                if not last:
                    # groups 4-5 written back early: the next step's pair 1
                    # reads rows 64-65 at ~+12us, before the pg==3 batch lands
                    h_writeback(a_nxt, 4, 6)
                else:
                    nc.sync.dma_start(
                        y_ap[t, :, 4 * NTP : 6 * NTP], hc[:, 1 + 4 * NTP : 1 + 6 * NTP]
                    )
            elif pg == 3:
                if not last:
                    h_writeback(a_nxt, 6, 8)
                    nc.sync.dma_start(y_ap[t, :, 4 * NTP :], hc[:, 1 + 4 * NTP :])
                else:
                    nc.sync.dma_start(y_ap[t, :, 6 * NTP :], hc[:, 1 + 6 * NTP :])

    for p in reversed(ctx_pools):
        p.__exit__(None, None, None)


def build_program(n_steps=S):
    nc = bacc.Bacc(
        "TRN2",
        target_bir_lowering=False,
        debug=False,
        enable_asserts=False,
        num_devices=N_CORES,
    )
    x_d = nc.dram_tensor("x", [n_steps, 2 * CIN, PADN], BF16, kind="ExternalInput")
    w_d = nc.dram_tensor("w", [128, 24 * 32], BF16, kind="ExternalInput")
    b_d = nc.dram_tensor("bias", [128, 4], F32, kind="ExternalInput")
    # y in padded raw layout: [t, 32*tile+ch, group*528 + 132*q + (1+w)], bf16
    y_d = nc.dram_tensor(
        "y", [n_steps, 128, GROUPS * NTP], BF16, kind="ExternalOutput"
    )
    z_d = nc.dram_tensor("z0", [64, PADN], BF16, kind="ExternalInput")
    with tile.TileContext(nc) as tc:
        build_kernel(
            nc, tc, x_d.ap(), w_d.ap(), b_d.ap(), y_d.ap(), z_d.ap(), n_steps
        )
    nc.compile()
    return nc


def pack_weights(Wd):
    """Wd [128, 64, 3, 3] f32 -> lhsT blocks [128, 24*32] bf16.

    Rows: 0-31 x center tap, 32-63 x shifted tap, 64-95 h center,
    96-127 h shifted.  Passes p<3: center (dy,0) + shift (dy,-1);
    p>=3: center (dy,+1), shift rows zero.
    """
    wp = np.zeros((128, 24, 32), np.float32)
    for g in range(4):
        Wg = Wd[g * 32 : (g + 1) * 32]  # [32(m), 64, 3, 3]
        for p in range(6):
            ky = p % 3
            blk = wp[:, g * 6 + p, :]
            if p < 3:
                blk[0:32, :] = Wg[:, 0:32, ky, 1].T     # x, dx=0 (center)
                blk[32:64, :] = Wg[:, 0:32, ky, 0].T    # x, dx=-1 (shifted)
                blk[64:96, :] = Wg[:, 32:64, ky, 1].T   # h, dx=0
                blk[96:128, :] = Wg[:, 32:64, ky, 0].T  # h, dx=-1
            else:
                blk[0:32, :] = Wg[:, 0:32, ky, 2].T     # x, dx=+1
                blk[64:96, :] = Wg[:, 32:64, ky, 2].T   # h, dx=+1
    return wp.reshape(128, 24 * 32).astype(ml_dtypes.bfloat16)


def pack_bias(bd):
    """bd [128] f32 -> [128, 4] f32 (partition p = 32*tile + ch)."""
    bp = np.zeros((128, 4), np.float32)
    for g in range(4):
        bp[:, g] = np.tile(bd[g * 32 : (g + 1) * 32], 4)
    return bp


def make_in_maps(x, Wf, bf, Wb, bb, n_steps=S):
    wpacks = [pack_weights(np.asarray(Wf, np.float32)),
              pack_weights(np.asarray(Wb, np.float32))]
    bpacks = [pack_bias(np.asarray(bf, np.float32)),
              pack_bias(np.asarray(bb, np.float32))]
    x = np.asarray(x, np.float32)
    in_maps = []
    for k in range(N_CORES):
        d, b = k // 4, k % 4
        xc = x[:n_steps, b] if d == 0 else x[::-1][:n_steps, b]
        xp = np.zeros((n_steps, 2 * CIN, HP, WP), ml_dtypes.bfloat16)
        xp[:, 0:CIN, 1 : H + 1, 1 : W + 1] = xc
        xp = xp.reshape(n_steps, 2 * CIN, PADN)
        # shifted copy: same stream one element later (zero border included)
        xp[:, CIN:, :].reshape(n_steps, -1)[:, 1:] = xp[:, 0:CIN, :].reshape(
            n_steps, -1
        )[:, :-1]
        in_maps.append(
            {
                "x": xp,
                "w": wpacks[d],
                "bias": bpacks[d],
                "z0": np.zeros((64, PADN), ml_dtypes.bfloat16),
            }
        )
    return in_maps


_CACHED_NC = None


def unpack_y(yk):
    """[S, 128, 8*4*132] bf16 padded raw layout -> [S, CO, H, W] f32."""
    yk = np.asarray(yk, np.float32).reshape(S, TPG, CO, GROUPS, 4, WP)[..., 1 : W + 1]
    return np.ascontiguousarray(yk.transpose(0, 2, 3, 1, 4, 5)).reshape(S, CO, H, W)


def kernel(x, Wf, bf, Wb, bb):
    global _CACHED_NC
    if _CACHED_NC is None:
        _CACHED_NC = build_program(S)
    nc = _CACHED_NC
    in_maps = make_in_maps(x, Wf, bf, Wb, bb)
    res = run_bass_kernel_spmd(nc, in_maps, core_ids=list(range(N_CORES)))
    out = np.empty((S, B, 2 * CO, H, W), np.float32)
    for k in range(N_CORES):
        d, b = k // 4, k % 4
        out[:, b, d * CO : (d + 1) * CO] = unpack_y(res.results[k]["y"])
    return out


if __name__ == "__main__":
    import jax

    jax.config.update("jax_platforms", "cpu")
    rng = np.random.default_rng(0)
    x = rng.standard_normal((S, B, CIN, H, W), np.float32)
    Wf = (rng.standard_normal((128, 64, 3, 3)) * 0.05).astype(np.float32)
    Wb = (rng.standard_normal((128, 64, 3, 3)) * 0.05).astype(np.float32)
    bf = np.zeros(128, np.float32)
    bb = np.zeros(128, np.float32)
    y = kernel(x, Wf, bf, Wb, bb)
    print("out", y.shape, y.dtype)


# revision 55
# speedup vs baseline: 1.2370x; 1.2370x over previous
"""Bidirectional ConvLSTM block for Trainium2 (Bass/Tile), 8-core SPMD.

Problem: x [S=16, B=4, Cin=32, H=128, W=128] f32, Wf/Wb [128, 64, 3, 3],
bf/bb [128].  Output [S, B, 2*Co=64, H, W]: forward ConvLSTM hidden states
concat backward ConvLSTM (run on time-reversed x, not re-flipped).

Sharding: 8 independent recurrences = 2 directions x 4 batch elements.
Core k runs direction d=k//4 on batch b=k%4.  No cross-core communication.

Per-core kernel design:
  - SBUF "act" tile [128 part, 130*132] bf16 (ping/pong): zero-padded
    (H+2)x(W+4) spatial plane per channel.
      partitions  0-31 : x_t   centered at col 1+w
      partitions 32-63 : x_t   at col 2+w (shifted right one column)
      partitions 64-95 : h_{t-1} centered
      partitions 96-127: h_{t-1} shifted
    The shifted copy lets one K=128 matmul cover two conv taps (center tap
    dx=dxo, shifted tap dx=dxo-1) at a single rhs column offset dxo+1.
    3x3 conv => 6 matmul passes per output tile; x and h grouped so step 0
    (h=0) runs K=64 x-only matmuls and never reads the h region (no init).
  - Col-tiled matmuls (tile_position (0,32j), mode 128x32): 4 concurrent
    M=32 matmuls per pass, one per spatial tile, each writing ITS gate's
    slice so each PSUM bank ends up gate-pure:
      bank[g] = [128 part = 32ch x 4 spatial tiles, 512] of gate g.
    All pointwise LSTM math then runs on full 128-partition tiles.
  - c state persistent fp32 [128, 4096]; h (bf16) written once into a
    padded 4-row-chunk buffer hc ([0,w0..w127,0,0,0] per chunk, one leading
    zero col).  hc serves: the y store (1 DMA per half step, bf16, host
    unpacks), the center write-back (3-dim SBUF->SBUF DMA per spatial tile
    b per half step), and the shifted write-back (same DMA reading hc at
    offset -1).  An "early" write-back of group 4's first rows keeps the
    next step's group-3 dy=+1 taps from stalling.
"""

import os
import sys

import numpy as np

for _p in ("/opt/trn_rl_repo", "/root/.axon_site/_ro/trn_rl_repo"):
    if os.path.isdir(_p) and _p not in sys.path:
        sys.path.insert(0, _p)

import ml_dtypes  # noqa: E402
import concourse.bass as bass  # noqa: E402,F401
import concourse.mybir as mybir  # noqa: E402
from concourse import bacc, tile  # noqa: E402
from concourse.bass_utils import run_bass_kernel_spmd  # noqa: E402

F32 = mybir.dt.float32
BF16 = mybir.dt.bfloat16
AF = mybir.ActivationFunctionType

S, B, CIN, H, W = 16, 4, 32, 128, 128
CO = 32
HP, WP = H + 2, W + 4          # 130 x 132 padded plane
PADN = HP * WP                 # 17160
NSP = H * W                    # 16384
NT = 512                       # spatial positions per matmul tile (4 rows)
NTP = 4 * WP                   # 528: padded 4-row chunk
TPG = 4                        # tiles per group (col-tiled together)
GROUPS = NSP // (NT * TPG)     # 8 groups per step; group = 16 image rows
N_CORES = 8

EXPLICIT_LDW = os.environ.get("BICLSTM_LDW", "0") == "1"


def mm_noldw(nc, out, lhsT, rhs, start, stop, tile_position):
    """nc.tensor.matmul clone with ldweights=False (weights pre-loaded)."""
    te = nc.tensor
    kd = {0}
    ifmap_ap = te.lower_ap(rhs.opt(kd), opt=False)
    weights_ap = te.lower_ap(lhsT.opt(kd), opt=False, for_matmul_weights=True)
    out_ap = te.lower_ap(out)
    return te.add_instruction(
        mybir.InstMatmult(
            name=te.bass.get_next_instruction_name(),
            replication_resolution=0,
            replication_shift_amnt=0,
            replication_num_rows=0,
            start_tensor_calc=start,
            stop_tensor_calc=stop,
            ins=[ifmap_ap, weights_ap],
            outs=[out_ap],
            perf_mode=None,
            is_transpose=None,
            ifmap_quant_offset=None,
            weights_quant_offset=None,
            bass_skip_group_check=True,
            tile_position=tile_position,
            tile_size=(128, 32),
            ldweights=False,
        )
    )


def build_kernel(nc, tc, x_ap, w_ap, b_ap, y_ap, z_ap, n_steps):
    ctx_pools = []

    def pool(**kw):
        p = tc.tile_pool(**kw)
        ctx_pools.append(p)
        return p.__enter__()

    const = pool(name="const", bufs=1)
    tmp = pool(name="tmp", bufs=3)
    psum = pool(name="psum", bufs=8, space="PSUM")

    # Persistent tiles
    a0 = const.tile([128, PADN], BF16, tag="act0")
    a1 = const.tile([128, PADN], BF16, tag="act1")
    acts = [a0, a1]
    ctile = const.tile([128, GROUPS * NT], F32, tag="c")
    wsb = const.tile([128, 24 * 32], BF16, tag="w")
    bsb = const.tile([128, 4], F32, tag="bias")
    hc = const.tile([128, GROUPS * NTP + 1], BF16, tag="hc")

    nc.sync.dma_start(wsb[:, :], w_ap)
    nc.sync.dma_start(bsb[:, :], b_ap)

    # --- one-time zero init.  Everything else is either host-padded (x),
    # written fully every step (h interior, hc chunks), or never read.
    # h_0 = 0: zero a0's whole h region via SWDGE DMA (parallel with the
    # sync-queue x/w loads); a1 only needs its h pad rows (fold covers rest).
    RS = 36 * WP  # first ~36 padded rows: what pair 0 reads
    nc.gpsimd.dma_start(a0[64:128, 0:RS], z_ap[:, 0:RS])
    nc.gpsimd.dma_start(a0[64:128, RS:], z_ap[:, RS:])
    nc.gpsimd.memset(a1[64:128, 0:WP], 0.0)
    nc.gpsimd.memset(a1[64:128, (HP - 1) * WP :], 0.0)
    hcb = hc[:, 1:].rearrange("p (k w) -> p k w", w=WP)
    nc.vector.memset(hc[:, 0:1], 0.0)
    nc.vector.memset(hcb[:, :, 0:1], 0.0)
    nc.gpsimd.memset(hcb[:, :, W + 1 :], 0.0)

    def load_x(t):
        # x arrives host-padded, center + shifted copies stacked: one DMA
        nc.sync.dma_start(acts[t % 2][0:64, :], x_ap[t])

    # step-0 x load split so the first pair's rows land first
    nc.sync.dma_start(a0[0:64, 0:RS], x_ap[0, :, 0:RS])
    nc.sync.dma_start(a0[0:64, RS:], x_ap[0, :, RS:])

    # h write-back: 4-row full-width (132) blocks, one 3-dim DMA per tile b.
    # Block (g, b) lands at padded plane rows 16g+4b+1 .. +5, cols 0..132.
    # hcv = center-aligned view, hcs = same data shifted one col right.
    hcv = hc[:, 1:].rearrange("p (g f) -> p g f", g=GROUPS)
    hcs = hc[:, 0 : GROUPS * NTP].rearrange("p (g f) -> p g f", g=GROUPS)
    hcq = hc[:, 1:].rearrange("p (g q w) -> p g q w", g=GROUPS, q=4)

    def h_views(a_nxt):
        vc = a_nxt[64:96, WP : WP + H * WP].rearrange(
            "p (g b f) -> p g b f", g=GROUPS, b=TPG
        )
        vs = a_nxt[96:128, WP : WP + H * WP].rearrange(
            "p (g b f) -> p g b f", g=GROUPS, b=TPG
        )
        return vc, vs

    def h_writeback(a_nxt, g0, g1, excl_b0=False):
        # interior rows 1..129 of the padded plane = 32 blocks of (4 x 132)
        vc, vs = h_views(a_nxt)
        for b in range(TPG):
            gg0 = g0 + 1 if (excl_b0 and b == 0) else g0
            nc.sync.dma_start(
                vc[:, gg0:g1, b, :], hcv[32 * b : 32 * b + 32, gg0:g1, :]
            )
            nc.sync.dma_start(
                vs[:, gg0:g1, b, :], hcs[32 * b : 32 * b + 32, gg0:g1, :]
            )

    for t in range(n_steps):
        a_cur = acts[t % 2]
        a_nxt = acts[(t + 1) % 2]
        ar_cur = a_cur.rearrange("p (r w) -> p r w", r=HP)
        if t + 1 < n_steps:
            load_x(t + 1)

        def pointwise(grp, zb):
            csl = ctile[:, grp * NT : (grp + 1) * NT]
            si = tmp.tile([128, NT], F32, tag="si", name=f"si{t}_{grp}")
            so = tmp.tile([128, NT], F32, tag="so", name=f"so{t}_{grp}")
            tg = tmp.tile([128, NT], F32, tag="tg", name=f"tg{t}_{grp}")
            nc.scalar.activation(si[:, :], zb[0][:, :], AF.Sigmoid, bias=bsb[:, 0:1])
            if t > 0:
                sf = tmp.tile([128, NT], F32, tag="sf", name=f"sf{t}_{grp}")
                nc.scalar.activation(sf[:, :], zb[1][:, :], AF.Sigmoid, bias=bsb[:, 1:2])
            nc.scalar.activation(so[:, :], zb[2][:, :], AF.Sigmoid, bias=bsb[:, 2:3])
            nc.scalar.activation(tg[:, :], zb[3][:, :], AF.Tanh, bias=bsb[:, 3:4])

            if t == 0:
                # c_{-1} = 0: c = sig(i)*tanh(g), no f*c term (ctile uninit)
                nc.vector.tensor_mul(csl, si[:, :], tg[:, :])
            else:
                t2 = tmp.tile([128, NT], F32, tag="t2", name=f"t2_{t}_{grp}")
                t3 = tmp.tile([128, NT], F32, tag="t3", name=f"t3_{t}_{grp}")
                # t3 only needs sf (2nd activation) - run it before t2 so the
                # DVE isn't blocked behind tanh_g on the critical chain
                nc.vector.tensor_mul(t3[:, :], sf[:, :], csl)
                nc.vector.tensor_mul(t2[:, :], si[:, :], tg[:, :])
                nc.vector.tensor_add(csl, t2[:, :], t3[:, :])

            tcn = tmp.tile([128, NT], F32, tag="tcn", name=f"tcn{t}_{grp}")
            nc.scalar.activation(tcn[:, :], csl, AF.Tanh)
            # h in bf16, written into the padded row-chunk buffer
            soq = so[:, :].rearrange("p (q w) -> p q w", q=4)
            tcq = tcn[:, :].rearrange("p (q w) -> p q w", q=4)
            nc.vector.tensor_mul(hcq[:, grp, :, 1 : W + 1], soq, tcq)

        # Group pairs: adjacent groups' matmuls interleaved so same-weight
        # matmuls are consecutive (better PE pipelining).
        for pg in range(GROUPS // 2):
            ga, gb = 2 * pg, 2 * pg + 1
            zbs = {
                grp: [
                    psum.tile([128, NT], F32, tag="z", name=f"z{t}_{grp}_{g}")
                    for g in range(4)
                ]
                for grp in (ga, gb)
            }
            for g in range(4):
                for p in range(6):
                    dy = (p % 3) - 1
                    dxo = 0 if p < 3 else 1
                    col = (g * 6 + p) * 32
                    lhsT = wsb[:, col : col + 32]
                    if EXPLICIT_LDW:
                        for j in range(TPG):
                            nc.tensor.ldweights(lhsT, tile_position=(0, 32 * j))
                    for j in range(TPG):
                        for grp in (ga, gb):
                            r0 = 16 * grp + 4 * j
                            rhs = ar_cur[
                                :, r0 + 1 + dy : r0 + 5 + dy, 1 + dxo : W + 1 + dxo
                            ]
                            if EXPLICIT_LDW:
                                mm_noldw(
                                    nc,
                                    zbs[grp][g][32 * j : 32 * j + 32, :],
                                    lhsT,
                                    rhs,
                                    start=(p == 0),
                                    stop=(p == 5),
                                    tile_position=(0, 32 * j),
                                )
                            else:
                                nc.tensor.matmul(
                                    zbs[grp][g][32 * j : 32 * j + 32, :],
                                    lhsT,
                                    rhs,
                                    start=(p == 0),
                                    stop=(p == 5),
                                    skip_group_check=True,
                                    tile_position=(0, 32 * j),
                                )
            pointwise(ga, zbs[ga])
            pointwise(gb, zbs[gb])
            last = t + 1 == n_steps
            if pg == 1:
                if not last:
                    h_writeback(a_nxt, 0, 4)
                nc.sync.dma_start(y_ap[t, :, 0 : 4 * NTP], hc[:, 1 : 1 + 4 * NTP])
            elif pg == 2:
                if not last:
                    # groups 4-5 written back early: the next step's pair 1
                    # reads rows 64-65 at ~+12us, before the pg==3 batch lands
                    h_writeback(a_nxt, 4, 6)
                else:
                    nc.sync.dma_start(
                        y_ap[t, :, 4 * NTP : 6 * NTP], hc[:, 1 + 4 * NTP : 1 + 6 * NTP]
                    )
            elif pg == 3:
                if not last:
                    h_writeback(a_nxt, 6, 8)
                    nc.sync.dma_start(y_ap[t, :, 4 * NTP :], hc[:, 1 + 4 * NTP :])
                else:
                    nc.sync.dma_start(y_ap[t, :, 6 * NTP :], hc[:, 1 + 6 * NTP :])

    for p in reversed(ctx_pools):
        p.__exit__(None, None, None)


def build_program(n_steps=S):
    nc = bacc.Bacc(
        "TRN2",
        target_bir_lowering=False,
        debug=False,
        enable_asserts=False,
        num_devices=N_CORES,
    )
    x_d = nc.dram_tensor("x", [n_steps, 2 * CIN, PADN], BF16, kind="ExternalInput")
    w_d = nc.dram_tensor("w", [128, 24 * 32], BF16, kind="ExternalInput")
    b_d = nc.dram_tensor("bias", [128, 4], F32, kind="ExternalInput")
    # y in padded raw layout: [t, 32*tile+ch, group*528 + 132*q + (1+w)], bf16
    y_d = nc.dram_tensor(
        "y", [n_steps, 128, GROUPS * NTP], BF16, kind="ExternalOutput"
    )
    z_d = nc.dram_tensor("z0", [64, PADN], BF16, kind="ExternalInput")
    with tile.TileContext(nc) as tc:
        build_kernel(
            nc, tc, x_d.ap(), w_d.ap(), b_d.ap(), y_d.ap(), z_d.ap(), n_steps
        )
    nc.compile()
    return nc


def pack_weights(Wd):
    """Wd [128, 64, 3, 3] f32 -> lhsT blocks [128, 24*32] bf16.

    Rows: 0-31 x center tap, 32-63 x shifted tap, 64-95 h center,
    96-127 h shifted.  Passes p<3: center (dy,0) + shift (dy,-1);
    p>=3: center (dy,+1), shift rows zero.
    """
    wp = np.zeros((128, 24, 32), np.float32)
    for g in range(4):
        Wg = Wd[g * 32 : (g + 1) * 32]  # [32(m), 64, 3, 3]
        for p in range(6):
            ky = p % 3
            blk = wp[:, g * 6 + p, :]
            if p < 3:
                blk[0:32, :] = Wg[:, 0:32, ky, 1].T     # x, dx=0 (center)
                blk[32:64, :] = Wg[:, 0:32, ky, 0].T    # x, dx=-1 (shifted)
                blk[64:96, :] = Wg[:, 32:64, ky, 1].T   # h, dx=0
                blk[96:128, :] = Wg[:, 32:64, ky, 0].T  # h, dx=-1
            else:
                blk[0:32, :] = Wg[:, 0:32, ky, 2].T     # x, dx=+1
                blk[64:96, :] = Wg[:, 32:64, ky, 2].T   # h, dx=+1
    return wp.reshape(128, 24 * 32).astype(ml_dtypes.bfloat16)


def pack_bias(bd):
    """bd [128] f32 -> [128, 4] f32 (partition p = 32*tile + ch)."""
    bp = np.zeros((128, 4), np.float32)
    for g in range(4):
        bp[:, g] = np.tile(bd[g * 32 : (g + 1) * 32], 4)
    return bp


def make_in_maps(x, Wf, bf, Wb, bb, n_steps=S):
    wpacks = [pack_weights(np.asarray(Wf, np.float32)),
              pack_weights(np.asarray(Wb, np.float32))]
    bpacks = [pack_bias(np.asarray(bf, np.float32)),
              pack_bias(np.asarray(bb, np.float32))]
    x = np.asarray(x, np.float32)
    in_maps = []
    for k in range(N_CORES):
        d, b = k // 4, k % 4
        xc = x[:n_steps, b] if d == 0 else x[::-1][:n_steps, b]
        xp = np.zeros((n_steps, 2 * CIN, HP, WP), ml_dtypes.bfloat16)
        xp[:, 0:CIN, 1 : H + 1, 1 : W + 1] = xc
        xp = xp.reshape(n_steps, 2 * CIN, PADN)
        # shifted copy: same stream one element later (zero border included)
        xp[:, CIN:, :].reshape(n_steps, -1)[:, 1:] = xp[:, 0:CIN, :].reshape(
            n_steps, -1
        )[:, :-1]
        in_maps.append(
            {
                "x": xp,
                "w": wpacks[d],
                "bias": bpacks[d],
                "z0": np.zeros((64, PADN), ml_dtypes.bfloat16),
            }
        )
    return in_maps


_CACHED_NC = None


def unpack_y(yk):
    """[S, 128, 8*4*132] bf16 padded raw layout -> [S, CO, H, W] f32."""
    yk = np.asarray(yk, np.float32).reshape(S, TPG, CO, GROUPS, 4, WP)[..., 1 : W + 1]
    return np.ascontiguousarray(yk.transpose(0, 2, 3, 1, 4, 5)).reshape(S, CO, H, W)


def kernel(x, Wf, bf, Wb, bb):
    global _CACHED_NC
    if _CACHED_NC is None:
        _CACHED_NC = build_program(S)
    nc = _CACHED_NC
    in_maps = make_in_maps(x, Wf, bf, Wb, bb)
    res = run_bass_kernel_spmd(nc, in_maps, core_ids=list(range(N_CORES)))
    out = np.empty((S, B, 2 * CO, H, W), np.float32)
    for k in range(N_CORES):
        d, b = k // 4, k % 4
        out[:, b, d * CO : (d + 1) * CO] = unpack_y(res.results[k]["y"])
    return out


if __name__ == "__main__":
    import jax

    jax.config.update("jax_platforms", "cpu")
    rng = np.random.default_rng(0)
    x = rng.standard_normal((S, B, CIN, H, W), np.float32)
    Wf = (rng.standard_normal((128, 64, 3, 3)) * 0.05).astype(np.float32)
    Wb = (rng.standard_normal((128, 64, 3, 3)) * 0.05).astype(np.float32)
    bf = np.zeros(128, np.float32)
    bb = np.zeros(128, np.float32)
    y = kernel(x, Wf, bf, Wb, bb)
    print("out", y.shape, y.dtype)
